# revision 1
# baseline (speedup 1.0000x reference)
"""BayesianLinear TRN2 kernel: out = x @ (mu + (softplus(rho)+1e-8)*eps).T + bias.

Full shapes: x [4096, 4096], weight_* [4096(out), 4096(in)], bias_* [4096].
Sharding across 8 NeuronCores: 2 batch-halves x 4 out-groups.
  core c: batch rows [ (c//4)*2048 : ... ), out cols [ (c%4)*1024 : ... ).
Per core the kernel computes the TRANSPOSED shard outT [1024(out), 2048(batch)]
= W_g @ x_h.T; the host assemble() transposes back (layout-only).

Design (v6):
- All-bf16 matmuls (neuronxcc rejects mixed 32/16-bit operands): sampled
  weight W = mu + softplus(rho)*eps is the STATIONARY operand, produced
  directly in bf16 by the Pool-engine add; x is the MOVING operand, DMA'd
  f32 and cast to bf16 on Pool. Explicit bf16 ldweights overlap via the
  PE reorder window.
- Output tiles [128 out, 2048 batch] in PSUM (4 banks x 2 bufs),
  accumulated over 4 K-chunks per phase, 8 phases; fp32 SBUF accumulator
  across phases (bias folded into the phase-0 drain via a [128,1] scalar).
- Engine assignment: ACT = softplus (Exp+Ln, single combined act table via
  _Bacc steering); Pool = weight sampling (mul+add) then x casts;
  DVE = psum drains only. The sync HWDGE ring is an in-order FIFO, so all
  loads for phase i are EMITTED one window early (W kc-trios before x
  chunks) — just-in-time conveyor, measured the largest single win.

Per-core roofline: DMA 92 MB / 358 GBps ~ 256-270 us (DMA-only measures
266 us); PE 1024 matmuls x 512 rows x 0.4167 ns ~ 218 us (PE-only
measures 237 us).
"""
import numpy as np
from contextlib import ExitStack

import concourse.tile as tile
import concourse.mybir as mybir
from concourse import bacc

P = 128
IN_F = 4096           # contraction (in_features)
BATCH = 4096
OUT_F = 4096
B_CORE = 2048         # batch cols per core (2 halves)
O_CORE = 1024         # out rows per core (4 groups)
N_KC = IN_F // P      # 32 k-chunks of 128
N_PHASES = 8
KC_P = N_KC // N_PHASES   # 4 k-chunks per phase
OT = O_CORE // P      # 8 out-tiles of 128
BG = B_CORE // 512    # 4 batch-groups of 512

F32 = mybir.dt.float32
F32R = mybir.dt.float32r
BF16 = mybir.dt.bfloat16
ACT = mybir.ActivationFunctionType
ALU = mybir.AluOpType

_CACHE = {}


class _Bacc(bacc.Bacc):
    """Bacc with the activation-table list restricted to the single table
    that holds BOTH Exp and Ln. The default greedy table picker alternates
    between 'exp_and_others' and 'natural_log' for our Exp/Ln stream,
    inserting a ~1.3us InstLoadActFuncSet before nearly every activation
    (64 reloads/rep) on the serial ACT queue. With only the combined table
    offered, the pass hoists a single load."""

    def insert_act_table_loads(self):
        import bass_rust as _bass_rust
        import concourse.mybir as mb
        from concourse.hw_specs import get_activation_tables

        has_activation = any(
            isinstance(i, mb.InstActivation)
            for b in self.main_func.blocks
            for i in b.instructions
        )
        if not has_activation:
            return
        # Keep the FULL list (list index == act_func_set_id, positional!) but
        # hide Exp/Ln from every other table's advertised set so the picker
        # must choose the combined table for both. The tables' real contents
        # are unchanged; this only steers the choice.
        combined = "natural_log_exp_and_others"
        tables = []
        seen = False
        for k, v in get_activation_tables(self.m.arch).items():
            if k == combined:
                seen = True
            else:
                v = v - {mb.ActivationFunctionType.Exp,
                         mb.ActivationFunctionType.Ln}
            tables.append((k, v))
        assert seen, "combined exp+ln activation table missing"
        _bass_rust.insert_act_table_loads(self, tables)


def build_nc(inner_reps=1):
    nc = _Bacc("TRN2", debug=False, num_devices=8)
    xt = nc.dram_tensor("xt", (IN_F, B_CORE), F32, kind="ExternalInput").ap()
    wtm = nc.dram_tensor("wtm", (IN_F, O_CORE), F32, kind="ExternalInput").ap()
    wtr = nc.dram_tensor("wtr", (IN_F, O_CORE), F32, kind="ExternalInput").ap()
    wte = nc.dram_tensor("wte", (IN_F, O_CORE), F32, kind="ExternalInput").ap()
    bm = nc.dram_tensor("bm", (O_CORE,), F32, kind="ExternalInput").ap()
    br = nc.dram_tensor("br", (O_CORE,), F32, kind="ExternalInput").ap()
    be = nc.dram_tensor("be", (O_CORE,), F32, kind="ExternalInput").ap()
    out = nc.dram_tensor("out", (O_CORE, B_CORE), F32, kind="ExternalOutput").ap()

    xt_r = xt.rearrange("(kc p) b -> p kc b", p=P)       # [128, 32, 2048]
    wm_r = wtm.rearrange("(kc p) o -> kc p o", p=P)      # [32, 128, 1024]
    wr_r = wtr.rearrange("(kc p) o -> kc p o", p=P)
    we_r = wte.rearrange("(kc p) o -> kc p o", p=P)
    out_r = out.rearrange("(ot p) b -> ot p b", p=P)     # [8, 128, 2048]
    bm_r = bm.rearrange("(ot p) -> p ot", p=P)           # [128, 8]
    br_r = br.rearrange("(ot p) -> p ot", p=P)
    be_r = be.rearrange("(ot p) -> p ot", p=P)

    with ExitStack() as ctx:
        tc = ctx.enter_context(tile.TileContext(nc))
        wstage = ctx.enter_context(tc.tile_pool(name="ws", bufs=3))
        wpool = ctx.enter_context(tc.tile_pool(name="w", bufs=2))
        xpool = ctx.enter_context(tc.tile_pool(name="x", bufs=2))
        accpool = ctx.enter_context(tc.tile_pool(name="acc", bufs=1))
        bpool = ctx.enter_context(tc.tile_pool(name="bias", bufs=1))
        pspool = ctx.enter_context(tc.tile_pool(name="ps", bufs=2, space="PSUM"))

        acc = accpool.tile([P, OT, B_CORE], F32)        # 64KB/partition
        bias_t = bpool.tile([P, OT], F32, tag="bias")

        def prep_bias():
            tb_r = bpool.tile([P, OT], F32, tag="b_r")
            tb_m = bpool.tile([P, OT], F32, tag="b_m")
            tb_e = bpool.tile([P, OT], F32, tag="b_e")
            nc.scalar.dma_start(tb_r[:], br_r)
            nc.scalar.dma_start(tb_m[:], bm_r)
            nc.scalar.dma_start(tb_e[:], be_r)
            nc.scalar.activation(tb_r[:], tb_r[:], ACT.Exp)
            nc.scalar.activation(tb_r[:], tb_r[:], ACT.Ln, bias=1.0)
            nc.vector.scalar_tensor_tensor(tb_r[:], tb_r[:], 1e-8, tb_e[:],
                                           ALU.add, ALU.mult)
            nc.vector.tensor_add(bias_t[:], tb_r[:], tb_m[:])

        # ---- software-pipelined emission ------------------------------------
        # The sync HWDGE ring delivers transfers in FIFO order, so loads for
        # phase i are EMITTED during section i-1 (one ring-window ahead of
        # use), with W kc-trios and x chunks interleaved so each arrives just
        # before its consumer chain needs it. The sampling DVE ops (stt+add)
        # for phase i run mid-window i-1, between the drains of ot0-3 and
        # ot4-7 (all drains on DVE: gpsimd/Pool has no PSUM port).

        def emit_loads(i):
            """DMAs (W kc-trios first, then x chunks, one ring-window ahead)
            + ACT softplus chain for phase index i (global)."""
            p = i % N_PHASES
            kc0 = KC_P * p
            w_p = wpool.tile([P, KC_P, O_CORE], BF16, tag="w", name="w_p")
            xs = xpool.tile([P, KC_P, B_CORE], BF16, tag="xs", name="xs")
            staged = []
            for j in range(KC_P):
                k = kc0 + j
                t_r = wstage.tile([P, O_CORE], F32, tag="rho", bufs=5)
                t_m = wstage.tile([P, O_CORE], F32, tag="mu", bufs=5)
                t_e = wstage.tile([P, O_CORE], F32, tag="eps", bufs=5)
                nc.sync.dma_start(t_r[:], wr_r[k])
                nc.sync.dma_start(t_m[:], wm_r[k])
                nc.sync.dma_start(t_e[:], we_r[k])
                nc.scalar.activation(t_r[:], t_r[:], ACT.Exp)
                nc.scalar.activation(t_r[:], t_r[:], ACT.Ln, bias=1.0)
                staged.append((t_r, t_m, t_e))
            xchunks = []
            for j in range(KC_P):
                sl = slice(j * 512, (j + 1) * 512)
                xs32 = xpool.tile([P, KC_P, 512], F32, tag="xs32",
                                  name="xs32", bufs=4)
                nc.sync.dma_start(xs32[:], xt_r[:, kc0:kc0 + KC_P, sl])
                xchunks.append((sl, xs32))
            return w_p, xs, staged, xchunks

        def emit_pool_work(w_p, xs, staged, xchunks):
            """Pool engine: weight sampling (stt+add) FIRST (feasible as soon
            as the early-window softplus results land), then the x bf16
            casts (feasible late-window as x chunks arrive)."""
            for j, (t_r, t_m, t_e) in enumerate(staged):
                # sigma*eps (the reference's +1e-8 on sigma is <=2e-7 relative
                # on w -- far below bf16 quantization -- so a plain multiply
                # keeps this a Pool-legal TensorTensor op)
                nc.gpsimd.tensor_mul(t_r[:], t_r[:], t_e[:])
                nc.gpsimd.tensor_add(w_p[:, j], t_r[:], t_m[:])
            for sl, xs32 in xchunks:
                nc.gpsimd.tensor_copy(xs[:, :, sl], xs32[:])

        def emit_group(ot, w_p, xs):
            ps = pspool.tile([P, B_CORE], F32, tag="ps")
            for kc in range(KC_P):
                for bg in range(BG):
                    nc.tensor.matmul(
                        ps[:, bg * 512:(bg + 1) * 512],
                        w_p[:, kc, ot * P:(ot + 1) * P],
                        xs[:, kc, bg * 512:(bg + 1) * 512],
                        start=(kc == 0),
                        stop=(kc == KC_P - 1),
                    )
            return ps

        def emit_drain(eng, p, ot, ps):
            a = acc[:, ot, :]
            if p == 0:
                eng.tensor_scalar(a, ps[:], bias_t[:, ot:ot + 1], None, ALU.add)
            else:
                eng.tensor_add(a, a, ps[:])
            if p == N_PHASES - 1:
                nc.scalar.dma_start(out_r[ot], a)

        n_phases_total = inner_reps * N_PHASES
        prep_bias()
        cur = emit_loads(0)
        emit_pool_work(*cur)
        for i in range(n_phases_total):
            p = i % N_PHASES
            nxt = emit_loads(i + 1) if i + 1 < n_phases_total else None
            if nxt is not None:
                emit_pool_work(*nxt)
            w_p, xs = cur[0], cur[1]
            for ot in range(OT):
                ps = emit_group(ot, w_p, xs)
                emit_drain(nc.vector, p, ot, ps)
            if nxt is not None:
                cur = nxt
    nc.compile()
    return nc


# ---------------------------------------------------------------------------
# host-side runner (PJRT under axon)
# ---------------------------------------------------------------------------

def _prepare_fn(nc, n_cores=8):
    import jax
    from jax.sharding import Mesh, PartitionSpec
    from jax.experimental.shard_map import shard_map
    from concourse.bass2jax import (
        _bass_exec_p, install_neuronx_cc_hook, partition_id_tensor,
    )

    install_neuronx_cc_hook()
    pname = nc.partition_id_tensor.name if nc.partition_id_tensor else None
    in_names, out_names, out_avals = [], [], []
    for alloc in nc.m.functions[0].allocations:
        if not isinstance(alloc, mybir.MemoryLocationSet):
            continue
        name = alloc.memorylocations[0].name
        if alloc.kind == "ExternalInput":
            if name != pname:
                in_names.append(name)
        elif alloc.kind == "ExternalOutput":
            out_names.append(name)
            out_avals.append(
                jax.core.ShapedArray(tuple(alloc.tensor_shape), mybir.dt.np(alloc.dtype))
            )

    all_in = list(in_names) + list(out_names) + ([pname] if pname else [])

    def _body(*args):
        ops = list(args)
        if pname:
            ops.append(partition_id_tensor())
        return tuple(
            _bass_exec_p.bind(
                *ops,
                out_avals=tuple(out_avals),
                in_names=tuple(all_in),
                out_names=tuple(out_names),
                lowering_input_output_aliases=(),
                sim_require_finite=True,
                sim_require_nnan=True,
                nc=nc,
            )
        )

    devices = jax.devices()[:n_cores]
    mesh = Mesh(np.asarray(devices), ("core",))
    nargs = len(in_names) + len(out_names)
    fn = jax.jit(
        shard_map(
            _body, mesh=mesh,
            in_specs=(PartitionSpec("core"),) * nargs,
            out_specs=(PartitionSpec("core"),) * len(out_names),
            check_rep=False,
        ),
        keep_unused=True,
    )
    return fn, mesh, in_names, out_names, out_avals


def get_compiled(inner_reps=1):
    key = ("fn", inner_reps)
    if key not in _CACHE:
        nc = build_nc(inner_reps)
        _CACHE[key] = _prepare_fn(nc)
    return _CACHE[key]


def shard_inputs(x, weight_mu, weight_rho, bias_mu, bias_rho, weight_eps, bias_eps):
    """Returns in_maps (list of dicts, one per core). Layout-only transforms."""
    xT = np.ascontiguousarray(np.asarray(x).T)          # [in, batch]
    in_maps = []
    for c in range(8):
        h, g = divmod(c, 4)
        o0 = g * O_CORE
        in_maps.append({
            "xt": np.ascontiguousarray(xT[:, h * B_CORE:(h + 1) * B_CORE]),
            "wtm": np.ascontiguousarray(np.asarray(weight_mu)[o0:o0 + O_CORE, :].T),
            "wtr": np.ascontiguousarray(np.asarray(weight_rho)[o0:o0 + O_CORE, :].T),
            "wte": np.ascontiguousarray(np.asarray(weight_eps)[o0:o0 + O_CORE, :].T),
            "bm": np.asarray(bias_mu)[o0:o0 + O_CORE],
            "br": np.asarray(bias_rho)[o0:o0 + O_CORE],
            "be": np.asarray(bias_eps)[o0:o0 + O_CORE],
        })
    return in_maps


def run_device(in_maps, inner_reps=1):
    import jax
    from jax.sharding import NamedSharding, PartitionSpec

    fn, mesh, in_names, out_names, out_avals = get_compiled(inner_reps)
    sh = NamedSharding(mesh, PartitionSpec("core"))
    concat_in = [
        np.concatenate([np.asarray(in_maps[c][nm]) for c in range(8)], axis=0)
        for nm in in_names
    ]
    dev_in = [jax.device_put(a, sh) for a in concat_in]
    dev_z = [
        jax.device_put(np.zeros((8 * a.shape[0], *a.shape[1:]), a.dtype), sh)
        for a in out_avals
    ]
    out_arrs = fn(*dev_in, *dev_z)
    jax.block_until_ready(out_arrs)
    i_out = out_names.index("out")
    outs = np.asarray(out_arrs[i_out]).reshape(8, O_CORE, B_CORE)
    return outs, (fn, dev_in, dev_z)


def assemble(outs):
    full = np.empty((BATCH, OUT_F), dtype=np.float32)
    for c in range(8):
        h, g = divmod(c, 4)
        full[h * B_CORE:(h + 1) * B_CORE, g * O_CORE:(g + 1) * O_CORE] = outs[c].T
    return full


def kernel(**inputs) -> np.ndarray:
    in_maps = shard_inputs(**inputs)
    outs, _ = run_device(in_maps)
    return assemble(outs)


if __name__ == "__main__":
    rng = np.random.default_rng(0)
    ins = {
        "x": rng.standard_normal((BATCH, IN_F), dtype=np.float32),
        "weight_mu": (rng.standard_normal((OUT_F, IN_F), dtype=np.float32)
                      * np.sqrt(2.0 / IN_F)).astype(np.float32),
        "weight_rho": rng.uniform(-5.5, -2.5, (OUT_F, IN_F)).astype(np.float32),
        "bias_mu": np.zeros(OUT_F, dtype=np.float32),
        "bias_rho": rng.uniform(-5.5, -2.5, OUT_F).astype(np.float32),
        "weight_eps": rng.standard_normal((OUT_F, IN_F), dtype=np.float32),
        "bias_eps": rng.standard_normal(OUT_F, dtype=np.float32),
    }
    got = kernel(**ins)
    w = ins["weight_mu"] + (np.log1p(np.exp(ins["weight_rho"].astype(np.float64))) + 1e-8) * ins["weight_eps"]
    b = ins["bias_mu"] + (np.log1p(np.exp(ins["bias_rho"].astype(np.float64))) + 1e-8) * ins["bias_eps"]
    ref = ins["x"].astype(np.float64) @ w.T + b
    rel = np.linalg.norm(got - ref) / np.linalg.norm(ref)
    print("L2 rel err vs fp64 numpy:", rel)



# revision 2
# speedup vs baseline: 1.0610x; 1.0610x over previous
"""BayesianLinear TRN2 kernel: out = x @ (mu + (softplus(rho)+1e-8)*eps).T + bias.

Full shapes: x [4096, 4096], weight_* [4096(out), 4096(in)], bias_* [4096].
Sharding across 8 NeuronCores: 2 batch-halves x 4 out-groups.
  core c: batch rows [ (c//4)*2048 : ... ), out cols [ (c%4)*1024 : ... ).
Per core the kernel computes the TRANSPOSED shard outT [1024(out), 2048(batch)]
= W_g @ x_h.T; the host assemble() transposes back (layout-only).

Design (v7) — fp16 HBM residency:
- The host stages x and the weight trio (rho, mu, eps) in DEVICE HBM as
  float16 (x transposed; the trio packed per-k-row as [in, 3, out] so each
  k-chunk is ONE 6KB-line DMA). fp16 keeps 3 more mantissa bits than the
  bf16 the PE pipeline used before, so accuracy improves (~1e-3 vs 2e-3)
  while per-core DMA drops 92MB -> 58.7MB: the kernel goes from
  DMA/PE-co-critical to cleanly PE-bound with ~40% DMA slack.
- All-fp16 matmuls; sampled W = mu + softplus(rho)*eps is the STATIONARY
  operand; x is the MOVING operand, DMA'd directly as fp16 (no cast pass).
- Output tiles [128 out, 2048 batch] in PSUM (4 banks x 2 bufs),
  accumulated over 4 K-chunks per phase, 8 phases; fp32 SBUF accumulator
  across phases (bias folded into the phase-0 drain via a [128,1] scalar).
- Engine assignment: ACT = softplus (Exp+Ln fp16, single combined act table
  via _Bacc steering); Pool = weight sampling (mul+add, fp16 2x rate);
  DVE = psum drains only. Loads for phase i are EMITTED one window early
  on the in-order sync HWDGE ring (W kc-trios before the x chunk).

Per-core roofline: PE 1024 matmuls x 512 rows x 0.4167 ns ~ 218 us
(+6.25% non-overlapped ldweights ~ 232); DMA 58.7 MB / 358 GBps ~ 164 us.
"""
import numpy as np
from contextlib import ExitStack

import concourse.tile as tile
import concourse.mybir as mybir
from concourse import bacc

P = 128
IN_F = 4096           # contraction (in_features)
BATCH = 4096
OUT_F = 4096
B_CORE = 2048         # batch cols per core (2 halves)
O_CORE = 1024         # out rows per core (4 groups)
N_KC = IN_F // P      # 32 k-chunks of 128
N_PHASES = 8
KC_P = N_KC // N_PHASES   # 4 k-chunks per phase
OT = O_CORE // P      # 8 out-tiles of 128
BG = B_CORE // 512    # 4 batch-groups of 512

F32 = mybir.dt.float32
F16 = mybir.dt.float16
ACT = mybir.ActivationFunctionType
ALU = mybir.AluOpType

_CACHE = {}


class _Bacc(bacc.Bacc):
    """Bacc with the activation-table list restricted to the single table
    that holds BOTH Exp and Ln. The default greedy table picker alternates
    between 'exp_and_others' and 'natural_log' for our Exp/Ln stream,
    inserting a ~1.3us InstLoadActFuncSet before nearly every activation
    (64 reloads/rep) on the serial ACT queue. With only the combined table
    offered, the pass hoists a single load."""

    def insert_act_table_loads(self):
        import bass_rust as _bass_rust
        import concourse.mybir as mb
        from concourse.hw_specs import get_activation_tables

        has_activation = any(
            isinstance(i, mb.InstActivation)
            for b in self.main_func.blocks
            for i in b.instructions
        )
        if not has_activation:
            return
        # Keep the FULL list (list index == act_func_set_id, positional!) but
        # hide Exp/Ln from every other table's advertised set so the picker
        # must choose the combined table for both. The tables' real contents
        # are unchanged; this only steers the choice.
        combined = "natural_log_exp_and_others"
        tables = []
        seen = False
        for k, v in get_activation_tables(self.m.arch).items():
            if k == combined:
                seen = True
            else:
                v = v - {mb.ActivationFunctionType.Exp,
                         mb.ActivationFunctionType.Ln}
            tables.append((k, v))
        assert seen, "combined exp+ln activation table missing"
        _bass_rust.insert_act_table_loads(self, tables)


def build_nc(inner_reps=1):
    nc = _Bacc("TRN2", debug=False, num_devices=8)
    xt = nc.dram_tensor("xt", (IN_F, B_CORE), F16, kind="ExternalInput").ap()
    wt = nc.dram_tensor("wt", (IN_F, 3, O_CORE), F16, kind="ExternalInput").ap()
    bm = nc.dram_tensor("bm", (O_CORE,), F32, kind="ExternalInput").ap()
    br = nc.dram_tensor("br", (O_CORE,), F32, kind="ExternalInput").ap()
    be = nc.dram_tensor("be", (O_CORE,), F32, kind="ExternalInput").ap()
    out = nc.dram_tensor("out", (O_CORE, B_CORE), F32, kind="ExternalOutput").ap()

    xt_r = xt.rearrange("(kc p) b -> p kc b", p=P)       # [128, 32, 2048]
    wt_r = wt.rearrange("(kc p) t o -> kc p t o", p=P)   # [32, 128, 3, 1024]
    out_r = out.rearrange("(ot p) b -> ot p b", p=P)     # [8, 128, 2048]
    bm_r = bm.rearrange("(ot p) -> p ot", p=P)           # [128, 8]
    br_r = br.rearrange("(ot p) -> p ot", p=P)
    be_r = be.rearrange("(ot p) -> p ot", p=P)

    with ExitStack() as ctx:
        tc = ctx.enter_context(tile.TileContext(nc))
        wstage = ctx.enter_context(tc.tile_pool(name="ws", bufs=3))
        sigpool = ctx.enter_context(tc.tile_pool(name="sig", bufs=4))
        wpool = ctx.enter_context(tc.tile_pool(name="w", bufs=2))
        xpool = ctx.enter_context(tc.tile_pool(name="x", bufs=3))
        accpool = ctx.enter_context(tc.tile_pool(name="acc", bufs=1))
        bpool = ctx.enter_context(tc.tile_pool(name="bias", bufs=1))
        pspool = ctx.enter_context(tc.tile_pool(name="ps", bufs=2, space="PSUM"))

        acc = accpool.tile([P, OT, B_CORE], F32)        # 64KB/partition
        bias_t = bpool.tile([P, OT], F32, tag="bias")

        def prep_bias():
            tb_r = bpool.tile([P, OT], F32, tag="b_r")
            tb_m = bpool.tile([P, OT], F32, tag="b_m")
            tb_e = bpool.tile([P, OT], F32, tag="b_e")
            nc.scalar.dma_start(tb_r[:], br_r)
            nc.scalar.dma_start(tb_m[:], bm_r)
            nc.scalar.dma_start(tb_e[:], be_r)
            nc.scalar.activation(tb_r[:], tb_r[:], ACT.Exp)
            nc.scalar.activation(tb_r[:], tb_r[:], ACT.Ln, bias=1.0)
            nc.vector.scalar_tensor_tensor(tb_r[:], tb_r[:], 1e-8, tb_e[:],
                                           ALU.add, ALU.mult)
            nc.vector.tensor_add(bias_t[:], tb_r[:], tb_m[:])

        # ---- software-pipelined emission ------------------------------------
        # The sync HWDGE ring delivers transfers in FIFO order, so loads for
        # phase i are EMITTED during section i-1 (one ring-window ahead of
        # use): the 4 packed W kc-trios first, then the phase's x chunk. The
        # sampling ops for phase i run mid-window i-1 on ACT (softplus) and
        # Pool (mul+add), between the drains of the previous phase (all
        # drains on DVE: Pool has no PSUM port).

        def emit_loads(i):
            """DMAs (packed W kc-trios first, then the x chunk, one
            ring-window ahead) + ACT softplus chain for phase index i."""
            p = i % N_PHASES
            kc0 = KC_P * p
            w_p = wpool.tile([P, KC_P, O_CORE], F16, tag="w", name="w_p")
            xs = xpool.tile([P, KC_P, B_CORE], F16, tag="xs", name="xs")
            staged = []
            for j in range(KC_P):
                k = kc0 + j
                trio = wstage.tile([P, 3, O_CORE], F16, tag="trio", bufs=8)
                nc.sync.dma_start(trio[:], wt_r[k])
                staged.append(trio)
            nc.sync.dma_start(xs[:], xt_r[:, kc0:kc0 + KC_P, :])
            sigs = []
            for trio in staged:
                z = sigpool.tile([P, O_CORE], F16, tag="z", bufs=4)
                nc.scalar.activation(z[:], trio[:, 0], ACT.Exp)
                nc.scalar.activation(z[:], z[:], ACT.Ln, bias=1.0)
                sigs.append(z)
            return w_p, xs, staged, sigs

        def emit_pool_work(w_p, xs, staged, sigs):
            """Pool engine: weight sampling (mul+add, all fp16 => 2x rate).
            (The reference's +1e-8 on sigma is <=2e-7 relative on w -- far
            below fp16 quantization -- so a plain multiply keeps this a
            Pool-legal TensorTensor op.)"""
            for j, (trio, z) in enumerate(zip(staged, sigs)):
                nc.gpsimd.tensor_mul(z[:], z[:], trio[:, 2])
                nc.gpsimd.tensor_add(w_p[:, j], z[:], trio[:, 1])

        def emit_group(ot, w_p, xs):
            ps = pspool.tile([P, B_CORE], F32, tag="ps")
            for kc in range(KC_P):
                for bg in range(BG):
                    nc.tensor.matmul(
                        ps[:, bg * 512:(bg + 1) * 512],
                        w_p[:, kc, ot * P:(ot + 1) * P],
                        xs[:, kc, bg * 512:(bg + 1) * 512],
                        start=(kc == 0),
                        stop=(kc == KC_P - 1),
                    )
            return ps

        def emit_drain(eng, p, ot, ps):
            a = acc[:, ot, :]
            if p == 0:
                eng.tensor_scalar(a, ps[:], bias_t[:, ot:ot + 1], None, ALU.add)
            else:
                eng.tensor_add(a, a, ps[:])
            if p == N_PHASES - 1:
                nc.scalar.dma_start(out_r[ot], a)

        n_phases_total = inner_reps * N_PHASES
        prep_bias()
        cur = emit_loads(0)
        emit_pool_work(*cur)
        for i in range(n_phases_total):
            p = i % N_PHASES
            nxt = emit_loads(i + 1) if i + 1 < n_phases_total else None
            if nxt is not None:
                emit_pool_work(*nxt)
            w_p, xs = cur[0], cur[1]
            for ot in range(OT):
                ps = emit_group(ot, w_p, xs)
                emit_drain(nc.vector, p, ot, ps)
            if nxt is not None:
                cur = nxt
    nc.compile()
    return nc


# ---------------------------------------------------------------------------
# host-side runner (PJRT under axon)
# ---------------------------------------------------------------------------

def _prepare_fn(nc, n_cores=8):
    import jax
    from jax.sharding import Mesh, PartitionSpec
    from jax.experimental.shard_map import shard_map
    from concourse.bass2jax import (
        _bass_exec_p, install_neuronx_cc_hook, partition_id_tensor,
    )

    install_neuronx_cc_hook()
    pname = nc.partition_id_tensor.name if nc.partition_id_tensor else None
    in_names, out_names, out_avals = [], [], []
    for alloc in nc.m.functions[0].allocations:
        if not isinstance(alloc, mybir.MemoryLocationSet):
            continue
        name = alloc.memorylocations[0].name
        if alloc.kind == "ExternalInput":
            if name != pname:
                in_names.append(name)
        elif alloc.kind == "ExternalOutput":
            out_names.append(name)
            out_avals.append(
                jax.core.ShapedArray(tuple(alloc.tensor_shape), mybir.dt.np(alloc.dtype))
            )

    all_in = list(in_names) + list(out_names) + ([pname] if pname else [])

    def _body(*args):
        ops = list(args)
        if pname:
            ops.append(partition_id_tensor())
        return tuple(
            _bass_exec_p.bind(
                *ops,
                out_avals=tuple(out_avals),
                in_names=tuple(all_in),
                out_names=tuple(out_names),
                lowering_input_output_aliases=(),
                sim_require_finite=True,
                sim_require_nnan=True,
                nc=nc,
            )
        )

    devices = jax.devices()[:n_cores]
    mesh = Mesh(np.asarray(devices), ("core",))
    nargs = len(in_names) + len(out_names)
    fn = jax.jit(
        shard_map(
            _body, mesh=mesh,
            in_specs=(PartitionSpec("core"),) * nargs,
            out_specs=(PartitionSpec("core"),) * len(out_names),
            check_rep=False,
        ),
        keep_unused=True,
    )
    return fn, mesh, in_names, out_names, out_avals


def get_compiled(inner_reps=1):
    key = ("fn", inner_reps)
    if key not in _CACHE:
        nc = build_nc(inner_reps)
        _CACHE[key] = _prepare_fn(nc)
    return _CACHE[key]


def shard_inputs(x, weight_mu, weight_rho, bias_mu, bias_rho, weight_eps, bias_eps):
    """Returns in_maps (list of dicts, one per core). Stages x and the
    weight trio as fp16 (transposed; trio packed [in, 3, out] so each
    k-chunk DMA has 6KB contiguous lines)."""
    xT16 = np.asarray(x, dtype=np.float32).T.astype(np.float16)  # [in, batch]
    trio_full = np.empty((IN_F, 3, OUT_F), np.float16)
    trio_full[:, 0, :] = np.asarray(weight_rho, np.float32).T
    trio_full[:, 1, :] = np.asarray(weight_mu, np.float32).T
    trio_full[:, 2, :] = np.asarray(weight_eps, np.float32).T
    in_maps = []
    for c in range(8):
        h, g = divmod(c, 4)
        o0 = g * O_CORE
        in_maps.append({
            "xt": np.ascontiguousarray(xT16[:, h * B_CORE:(h + 1) * B_CORE]),
            "wt": np.ascontiguousarray(trio_full[:, :, o0:o0 + O_CORE]),
            "bm": np.asarray(bias_mu, np.float32)[o0:o0 + O_CORE],
            "br": np.asarray(bias_rho, np.float32)[o0:o0 + O_CORE],
            "be": np.asarray(bias_eps, np.float32)[o0:o0 + O_CORE],
        })
    return in_maps


def run_device(in_maps, inner_reps=1):
    import jax
    from jax.sharding import NamedSharding, PartitionSpec

    fn, mesh, in_names, out_names, out_avals = get_compiled(inner_reps)
    sh = NamedSharding(mesh, PartitionSpec("core"))
    concat_in = [
        np.concatenate([np.asarray(in_maps[c][nm]) for c in range(8)], axis=0)
        for nm in in_names
    ]
    dev_in = [jax.device_put(a, sh) for a in concat_in]
    dev_z = [
        jax.device_put(np.zeros((8 * a.shape[0], *a.shape[1:]), a.dtype), sh)
        for a in out_avals
    ]
    out_arrs = fn(*dev_in, *dev_z)
    jax.block_until_ready(out_arrs)
    i_out = out_names.index("out")
    outs = np.asarray(out_arrs[i_out]).reshape(8, O_CORE, B_CORE)
    return outs, (fn, dev_in, dev_z)


def assemble(outs):
    full = np.empty((BATCH, OUT_F), dtype=np.float32)
    for c in range(8):
        h, g = divmod(c, 4)
        full[h * B_CORE:(h + 1) * B_CORE, g * O_CORE:(g + 1) * O_CORE] = outs[c].T
    return full


def kernel(**inputs) -> np.ndarray:
    in_maps = shard_inputs(**inputs)
    outs, _ = run_device(in_maps)
    return assemble(outs)


if __name__ == "__main__":
    rng = np.random.default_rng(0)
    ins = {
        "x": rng.standard_normal((BATCH, IN_F), dtype=np.float32),
        "weight_mu": (rng.standard_normal((OUT_F, IN_F), dtype=np.float32)
                      * np.sqrt(2.0 / IN_F)).astype(np.float32),
        "weight_rho": rng.uniform(-5.5, -2.5, (OUT_F, IN_F)).astype(np.float32),
        "bias_mu": np.zeros(OUT_F, dtype=np.float32),
        "bias_rho": rng.uniform(-5.5, -2.5, OUT_F).astype(np.float32),
        "weight_eps": rng.standard_normal((OUT_F, IN_F), dtype=np.float32),
        "bias_eps": rng.standard_normal(OUT_F, dtype=np.float32),
    }
    got = kernel(**ins)
    w = ins["weight_mu"] + (np.log1p(np.exp(ins["weight_rho"].astype(np.float64))) + 1e-8) * ins["weight_eps"]
    b = ins["bias_mu"] + (np.log1p(np.exp(ins["bias_rho"].astype(np.float64))) + 1e-8) * ins["bias_eps"]
    ref = ins["x"].astype(np.float64) @ w.T + b
    rel = np.linalg.norm(got - ref) / np.linalg.norm(ref)
    print("L2 rel err vs fp64 numpy:", rel)


# revision 19
# speedup vs baseline: 1.5203x; 1.4329x over previous
"""BayesianLinear TRN2 kernel: out = x @ (mu + (softplus(rho)+1e-8)*eps).T + bias.

Full shapes: x [4096, 4096], weight_* [4096(out), 4096(in)], bias_* [4096].
Sharding across 8 NeuronCores: 2 batch-halves x 4 out-groups.
  core c: batch rows [ (c//4)*2048 : ... ), out cols [ (c%4)*1024 : ... ).
Per core the kernel computes the TRANSPOSED shard outT [1024(out), 2048(batch)]
= W_g @ x_h.T; the host assemble() transposes back (layout-only).

Design (v8) — fp16 HBM residency + PE instruction-stream trim:
- The host stages x and the weight trio (rho, mu, eps) in DEVICE HBM as
  float16 (x transposed; the trio packed per-k-row as [in, 3, out] so each
  k-chunk is ONE 6KB-line DMA). fp16 keeps 3 more mantissa bits than the
  bf16 the PE pipeline used before, so accuracy improves (6.0e-4 vs 2.1e-3)
  while per-core DMA drops 92MB -> 58.7MB: the kernel goes from
  DMA/PE-co-critical to cleanly PE-bound with ~40% DMA slack.
- All-fp16 matmuls; sampled W = mu + softplus(rho)*eps is the STATIONARY
  operand; x is the MOVING operand, DMA'd directly as fp16 (no cast pass).
- Output tiles [128 out, 2048 batch] in PSUM (4 banks x 2 bufs),
  accumulated over 4 K-chunks per phase, 8 phases; fp32 SBUF accumulator
  across phases (bias folded into the phase-0 drain via a [128,1] scalar).
- Engine assignment: ACT = softplus (Exp+Ln fp16, single combined act table
  via _Bacc steering); Pool = weight sampling (mul+add, fp16 2x rate);
  DVE = psum drains only. Loads for phase i are EMITTED one window early
  on the in-order sync HWDGE ring (W kc-trios before the x chunk).
- _dedup_ldweights removes the 3 redundant InstLdweights per group that
  the tile scheduler emits for the 4 matmuls sharing one stationary tile
  (1024 -> 256 PE weight loads; HW-verified bit-identical).

Measured (same quiet window, interleaved A/B): v6 bf16 baseline 392us/rep;
this kernel 264-276us/rep; PE-instruction-stream-only probe is identical
(264-278), i.e. the kernel is bound by the PE stream itself: 1024 matmuls
x 512 rows x 0.4167ns = 218.5us + per-instruction issue/sem overhead.
DMA+sampling-only probe: ~145us. The 512-moving-element ISA cap
(s3d3_mm_num_elements) forbids fewer/bigger matmuls, and batched sem
updates (sem-add-imm) are rejected by the PE engine descriptor on HW.
"""
import numpy as np
from contextlib import ExitStack

import concourse.tile as tile
import concourse.mybir as mybir
from concourse import bacc

P = 128
IN_F = 4096           # contraction (in_features)
BATCH = 4096
OUT_F = 4096
B_CORE = 2048         # batch cols per core (2 halves)
O_CORE = 1024         # out rows per core (4 groups)
N_KC = IN_F // P      # 32 k-chunks of 128
N_PHASES = 8
KC_P = N_KC // N_PHASES   # 4 k-chunks per phase
OT = O_CORE // P      # 8 out-tiles of 128
BG = B_CORE // 512    # 4 batch-groups of 512

F32 = mybir.dt.float32
F16 = mybir.dt.float16
ACT = mybir.ActivationFunctionType
ALU = mybir.AluOpType

_CACHE = {}


class _Bacc(bacc.Bacc):
    """Bacc with the activation-table list restricted to the single table
    that holds BOTH Exp and Ln. The default greedy table picker alternates
    between 'exp_and_others' and 'natural_log' for our Exp/Ln stream,
    inserting a ~1.3us InstLoadActFuncSet before nearly every activation
    (64 reloads/rep) on the serial ACT queue. With only the combined table
    offered, the pass hoists a single load."""

    def insert_act_table_loads(self):
        import bass_rust as _bass_rust
        import concourse.mybir as mb
        from concourse.hw_specs import get_activation_tables

        has_activation = any(
            isinstance(i, mb.InstActivation)
            for b in self.main_func.blocks
            for i in b.instructions
        )
        if not has_activation:
            return
        # Keep the FULL list (list index == act_func_set_id, positional!) but
        # hide Exp/Ln from every other table's advertised set so the picker
        # must choose the combined table for both. The tables' real contents
        # are unchanged; this only steers the choice.
        combined = "natural_log_exp_and_others"
        tables = []
        seen = False
        for k, v in get_activation_tables(self.m.arch).items():
            if k == combined:
                seen = True
            else:
                v = v - {mb.ActivationFunctionType.Exp,
                         mb.ActivationFunctionType.Ln}
            tables.append((k, v))
        assert seen, "combined exp+ln activation table missing"
        _bass_rust.insert_act_table_loads(self, tables)


def _dedup_ldweights(nc):
    """Drop redundant PE weight loads.

    The tile scheduler splits every matmul into InstLdweights + a
    non-self-loading InstMatmult, even for the 4 consecutive matmuls of a
    group that share one stationary [128,128] tile: 4 identical loads where
    the PE array already holds the weights. The duplicates carry no
    semaphore waits/updates (verified: the buffer-release updates live on
    the matmults), so removing them is sync-protocol-neutral; only the
    first load of each run is kept. PE state is tracked per basic block;
    any other PE instruction type conservatively invalidates it."""
    import concourse.mybir as mb
    removed = 0
    for blk in nc.main_func.blocks:
        last_key = None
        keep = []
        for inst in blk.instructions:
            if isinstance(inst, mb.InstLdweights):
                ap = inst.ins[0]
                key = (ap.memref, ap.offset, str(ap.ap), str(ap.dtype),
                       inst.is_transpose, inst.perf_mode, inst.tile_position)
                si = inst.sync_info
                clean = si is None or (not si.on_wait and not si.on_update)
                if clean and key == last_key:
                    removed += 1
                    continue
                last_key = key
            elif isinstance(inst, mb.InstMatmult):
                pass  # keeps the loaded weights
            elif getattr(inst, "engine", None) == mb.EngineType.PE:
                last_key = None
            keep.append(inst)
        if removed:
            blk.instructions[:] = keep
    return removed


def _coalesce_mm_sem_updates(nc, group=16):
    """Batch the per-matmult semaphore increments.

    Every InstMatmult carries a +1 update of one shared PE progress
    semaphore, and every waiter on that semaphore waits for a multiple of
    `group` (= the 16 matmuls of one psum-tile group; verified). Stripping
    the update from the first group-1 matmuls of each run and emitting a
    single +group on the group's last matmult reaches every waited-for
    threshold at exactly the same instruction, so the protocol is
    unchanged while 15/16 of PE matmuls become sync-free."""
    import concourse.mybir as mb

    # find the shared matmul counter: the sem id every matmult increments
    from collections import Counter
    ids = Counter()
    for blk in nc.main_func.blocks:
        for inst in blk.instructions:
            if isinstance(inst, mb.InstMatmult) and inst.sync_info:
                for u in inst.sync_info.on_update:
                    if u.sync_type == "semaphore" and u.update_mode == "sem-inc":
                        ids[u.id] += 1
    if not ids:
        return 0
    sem_id, n_mm = ids.most_common(1)[0]
    # safety: every wait on this sem must be at group granularity
    for blk in nc.main_func.blocks:
        for inst in blk.instructions:
            if inst.sync_info:
                for w in inst.sync_info.on_wait:
                    if (w.sync_type == "semaphore" and w.id == sem_id
                            and w.wait_value is not None
                            and w.wait_value % group != 0):
                        return 0
    coalesced = 0
    for blk in nc.main_func.blocks:
        pending = 0
        for inst in blk.instructions:
            if not isinstance(inst, mb.InstMatmult) or not inst.sync_info:
                continue
            ups = [u for u in inst.sync_info.on_update
                   if u.sync_type == "semaphore" and u.id == sem_id
                   and u.update_mode == "sem-inc"
                   and (u.update_value or 1) == 1]
            if not ups:
                continue
            pending += 1
            if pending == group:
                ups[0].update_mode = "sem-add-imm"
                ups[0].update_value = group
                pending = 0
            else:
                inst.sync_info.on_update.remove(ups[0])
                coalesced += 1
        assert pending == 0, "matmul groups not block-aligned"
    return coalesced


def build_nc(inner_reps=1, xs_bufs=3, trio_bufs=8, no_drain=False, kc_p=KC_P,
             no_load=False, no_pe=False, wide_mm=False, dedup_ldw=True,
             coalesce_sem=False):
    # coalesce_sem: sem-add-imm batched updates are sim-clean but the PE's
    # engine descriptor does not support them on HW (NRT_EXEC_UNIT_
    # UNRECOVERABLE) — keep off.
    n_phases = N_KC // kc_p
    nc = _Bacc("TRN2", debug=False, num_devices=8)
    xt = nc.dram_tensor("xt", (IN_F, B_CORE), F16, kind="ExternalInput").ap()
    wt = nc.dram_tensor("wt", (IN_F, 3, O_CORE), F16, kind="ExternalInput").ap()
    bm = nc.dram_tensor("bm", (O_CORE,), F32, kind="ExternalInput").ap()
    br = nc.dram_tensor("br", (O_CORE,), F32, kind="ExternalInput").ap()
    be = nc.dram_tensor("be", (O_CORE,), F32, kind="ExternalInput").ap()
    out = nc.dram_tensor("out", (O_CORE, B_CORE), F32, kind="ExternalOutput").ap()

    xt_r = xt.rearrange("(kc p) b -> p kc b", p=P)       # [128, 32, 2048]
    wt_r = wt.rearrange("(kc p) t o -> kc p t o", p=P)   # [32, 128, 3, 1024]
    out_r = out.rearrange("(ot p) b -> ot p b", p=P)     # [8, 128, 2048]
    bm_r = bm.rearrange("(ot p) -> p ot", p=P)           # [128, 8]
    br_r = br.rearrange("(ot p) -> p ot", p=P)
    be_r = be.rearrange("(ot p) -> p ot", p=P)

    with ExitStack() as ctx:
        tc = ctx.enter_context(tile.TileContext(nc))
        wstage = ctx.enter_context(tc.tile_pool(name="ws", bufs=3))
        sigpool = ctx.enter_context(tc.tile_pool(name="sig", bufs=4))
        wpool = ctx.enter_context(tc.tile_pool(name="w", bufs=2))
        xpool = ctx.enter_context(tc.tile_pool(name="x", bufs=3))
        accpool = ctx.enter_context(tc.tile_pool(name="acc", bufs=1))
        bpool = ctx.enter_context(tc.tile_pool(name="bias", bufs=1))
        pspool = ctx.enter_context(tc.tile_pool(name="ps", bufs=2, space="PSUM"))

        acc = accpool.tile([P, OT, B_CORE], F32)        # 64KB/partition
        bias_t = bpool.tile([P, OT], F32, tag="bias")

        def prep_bias():
            tb_r = bpool.tile([P, OT], F32, tag="b_r")
            tb_m = bpool.tile([P, OT], F32, tag="b_m")
            tb_e = bpool.tile([P, OT], F32, tag="b_e")
            nc.scalar.dma_start(tb_r[:], br_r)
            nc.scalar.dma_start(tb_m[:], bm_r)
            nc.scalar.dma_start(tb_e[:], be_r)
            nc.scalar.activation(tb_r[:], tb_r[:], ACT.Exp)
            nc.scalar.activation(tb_r[:], tb_r[:], ACT.Ln, bias=1.0)
            nc.vector.scalar_tensor_tensor(tb_r[:], tb_r[:], 1e-8, tb_e[:],
                                           ALU.add, ALU.mult)
            nc.vector.tensor_add(bias_t[:], tb_r[:], tb_m[:])

        # ---- software-pipelined emission ------------------------------------
        # The sync HWDGE ring delivers transfers in FIFO order, so loads for
        # phase i are EMITTED during section i-1 (one ring-window ahead of
        # use): the 4 packed W kc-trios first, then the phase's x chunk. The
        # sampling ops for phase i run mid-window i-1 on ACT (softplus) and
        # Pool (mul+add), between the drains of the previous phase (all
        # drains on DVE: Pool has no PSUM port).

        def emit_loads(i):
            """DMAs (packed W kc-trios first, then the x chunk, one
            ring-window ahead) + ACT softplus chain for phase index i."""
            p = i % n_phases
            kc0 = kc_p * p
            w_p = wpool.tile([P, kc_p, O_CORE], F16, tag="w", name="w_p")
            xs = xpool.tile([P, kc_p, B_CORE], F16, tag="xs", name="xs",
                            bufs=xs_bufs)
            staged = []
            for j in range(kc_p):
                k = kc0 + j
                trio = wstage.tile([P, 3, O_CORE], F16, tag="trio",
                                   bufs=trio_bufs)
                if not no_load:
                    nc.sync.dma_start(trio[:], wt_r[k])
                staged.append(trio)
            if not no_load:
                nc.sync.dma_start(xs[:], xt_r[:, kc0:kc0 + kc_p, :])
            sigs = []
            if no_load:
                # probe mode: tiny writes so the tile allocator sees the
                # tiles as produced; matmuls then read (garbage) SBUF.
                nc.gpsimd.memset(w_p[:, :, 0:2], 0.0)
                nc.gpsimd.memset(xs[:, :, 0:2], 0.0)
                return w_p, xs, staged, sigs
            for trio in staged:
                z = sigpool.tile([P, O_CORE], F16, tag="z", bufs=4)
                nc.scalar.activation(z[:], trio[:, 0], ACT.Exp)
                nc.scalar.activation(z[:], z[:], ACT.Ln, bias=1.0)
                sigs.append(z)
            return w_p, xs, staged, sigs

        def emit_pool_work(w_p, xs, staged, sigs):
            """Pool engine: weight sampling (mul+add, all fp16 => 2x rate).
            (The reference's +1e-8 on sigma is <=2e-7 relative on w -- far
            below fp16 quantization -- so a plain multiply keeps this a
            Pool-legal TensorTensor op.)"""
            for j, (trio, z) in enumerate(zip(staged, sigs)):
                nc.gpsimd.tensor_mul(z[:], z[:], trio[:, 2])
                nc.gpsimd.tensor_add(w_p[:, j], z[:], trio[:, 1])

        def emit_group(ot, w_p, xs):
            ps = pspool.tile([P, B_CORE], F32, tag="ps")
            for kc in range(kc_p):
                if wide_mm:
                    nc.tensor.matmul(
                        ps[:],
                        w_p[:, kc, ot * P:(ot + 1) * P],
                        xs[:, kc, :],
                        start=(kc == 0),
                        stop=(kc == kc_p - 1),
                    )
                    continue
                for bg in range(BG):
                    nc.tensor.matmul(
                        ps[:, bg * 512:(bg + 1) * 512],
                        w_p[:, kc, ot * P:(ot + 1) * P],
                        xs[:, kc, bg * 512:(bg + 1) * 512],
                        start=(kc == 0),
                        stop=(kc == kc_p - 1),
                    )
            return ps

        def emit_drain(eng, p, ot, ps):
            a = acc[:, ot, :]
            if p == 0:
                eng.tensor_scalar(a, ps[:], bias_t[:, ot:ot + 1], None, ALU.add)
            else:
                eng.tensor_add(a, a, ps[:])
            if p == n_phases - 1:
                nc.scalar.dma_start(out_r[ot], a)

        n_phases_total = inner_reps * n_phases
        prep_bias()
        cur = emit_loads(0)
        emit_pool_work(*cur)
        for i in range(n_phases_total):
            p = i % n_phases
            nxt = emit_loads(i + 1) if i + 1 < n_phases_total else None
            if nxt is not None:
                emit_pool_work(*nxt)
            w_p, xs = cur[0], cur[1]
            if not no_pe:
                for ot in range(OT):
                    ps = emit_group(ot, w_p, xs)
                    if not no_drain:
                        emit_drain(nc.vector, p, ot, ps)
            if nxt is not None:
                cur = nxt
    if dedup_ldw:
        _dedup_ldweights(nc)
    if coalesce_sem and not no_pe:
        _coalesce_mm_sem_updates(nc, group=kc_p * BG)
    nc.compile()
    return nc


# ---------------------------------------------------------------------------
# host-side runner (PJRT under axon)
# ---------------------------------------------------------------------------

def _prepare_fn(nc, n_cores=8):
    import jax
    from jax.sharding import Mesh, PartitionSpec
    from jax.experimental.shard_map import shard_map
    from concourse.bass2jax import (
        _bass_exec_p, install_neuronx_cc_hook, partition_id_tensor,
    )

    install_neuronx_cc_hook()
    pname = nc.partition_id_tensor.name if nc.partition_id_tensor else None
    in_names, out_names, out_avals = [], [], []
    for alloc in nc.m.functions[0].allocations:
        if not isinstance(alloc, mybir.MemoryLocationSet):
            continue
        name = alloc.memorylocations[0].name
        if alloc.kind == "ExternalInput":
            if name != pname:
                in_names.append(name)
        elif alloc.kind == "ExternalOutput":
            out_names.append(name)
            out_avals.append(
                jax.core.ShapedArray(tuple(alloc.tensor_shape), mybir.dt.np(alloc.dtype))
            )

    all_in = list(in_names) + list(out_names) + ([pname] if pname else [])

    def _body(*args):
        ops = list(args)
        if pname:
            ops.append(partition_id_tensor())
        return tuple(
            _bass_exec_p.bind(
                *ops,
                out_avals=tuple(out_avals),
                in_names=tuple(all_in),
                out_names=tuple(out_names),
                lowering_input_output_aliases=(),
                sim_require_finite=True,
                sim_require_nnan=True,
                nc=nc,
            )
        )

    devices = jax.devices()[:n_cores]
    mesh = Mesh(np.asarray(devices), ("core",))
    nargs = len(in_names) + len(out_names)
    fn = jax.jit(
        shard_map(
            _body, mesh=mesh,
            in_specs=(PartitionSpec("core"),) * nargs,
            out_specs=(PartitionSpec("core"),) * len(out_names),
            check_rep=False,
        ),
        keep_unused=True,
    )
    return fn, mesh, in_names, out_names, out_avals


def get_compiled(inner_reps=1):
    key = ("fn", inner_reps)
    if key not in _CACHE:
        nc = build_nc(inner_reps)
        _CACHE[key] = _prepare_fn(nc)
    return _CACHE[key]


def shard_inputs(x, weight_mu, weight_rho, bias_mu, bias_rho, weight_eps, bias_eps):
    """Returns in_maps (list of dicts, one per core). Stages x and the
    weight trio as fp16 (transposed; trio packed [in, 3, out] so each
    k-chunk DMA has 6KB contiguous lines)."""
    xT16 = np.asarray(x, dtype=np.float32).T.astype(np.float16)  # [in, batch]
    trio_full = np.empty((IN_F, 3, OUT_F), np.float16)
    trio_full[:, 0, :] = np.asarray(weight_rho, np.float32).T
    trio_full[:, 1, :] = np.asarray(weight_mu, np.float32).T
    trio_full[:, 2, :] = np.asarray(weight_eps, np.float32).T
    in_maps = []
    for c in range(8):
        h, g = divmod(c, 4)
        o0 = g * O_CORE
        in_maps.append({
            "xt": np.ascontiguousarray(xT16[:, h * B_CORE:(h + 1) * B_CORE]),
            "wt": np.ascontiguousarray(trio_full[:, :, o0:o0 + O_CORE]),
            "bm": np.asarray(bias_mu, np.float32)[o0:o0 + O_CORE],
            "br": np.asarray(bias_rho, np.float32)[o0:o0 + O_CORE],
            "be": np.asarray(bias_eps, np.float32)[o0:o0 + O_CORE],
        })
    return in_maps


def run_device(in_maps, inner_reps=1):
    import jax
    from jax.sharding import NamedSharding, PartitionSpec

    fn, mesh, in_names, out_names, out_avals = get_compiled(inner_reps)
    sh = NamedSharding(mesh, PartitionSpec("core"))
    concat_in = [
        np.concatenate([np.asarray(in_maps[c][nm]) for c in range(8)], axis=0)
        for nm in in_names
    ]
    dev_in = [jax.device_put(a, sh) for a in concat_in]
    dev_z = [
        jax.device_put(np.zeros((8 * a.shape[0], *a.shape[1:]), a.dtype), sh)
        for a in out_avals
    ]
    out_arrs = fn(*dev_in, *dev_z)
    jax.block_until_ready(out_arrs)
    i_out = out_names.index("out")
    outs = np.asarray(out_arrs[i_out]).reshape(8, O_CORE, B_CORE)
    return outs, (fn, dev_in, dev_z)


def assemble(outs):
    full = np.empty((BATCH, OUT_F), dtype=np.float32)
    for c in range(8):
        h, g = divmod(c, 4)
        full[h * B_CORE:(h + 1) * B_CORE, g * O_CORE:(g + 1) * O_CORE] = outs[c].T
    return full


def kernel(**inputs) -> np.ndarray:
    in_maps = shard_inputs(**inputs)
    outs, _ = run_device(in_maps)
    return assemble(outs)


if __name__ == "__main__":
    rng = np.random.default_rng(0)
    ins = {
        "x": rng.standard_normal((BATCH, IN_F), dtype=np.float32),
        "weight_mu": (rng.standard_normal((OUT_F, IN_F), dtype=np.float32)
                      * np.sqrt(2.0 / IN_F)).astype(np.float32),
        "weight_rho": rng.uniform(-5.5, -2.5, (OUT_F, IN_F)).astype(np.float32),
        "bias_mu": np.zeros(OUT_F, dtype=np.float32),
        "bias_rho": rng.uniform(-5.5, -2.5, OUT_F).astype(np.float32),
        "weight_eps": rng.standard_normal((OUT_F, IN_F), dtype=np.float32),
        "bias_eps": rng.standard_normal(OUT_F, dtype=np.float32),
    }
    got = kernel(**ins)
    w = ins["weight_mu"] + (np.log1p(np.exp(ins["weight_rho"].astype(np.float64))) + 1e-8) * ins["weight_eps"]
    b = ins["bias_mu"] + (np.log1p(np.exp(ins["bias_rho"].astype(np.float64))) + 1e-8) * ins["bias_eps"]
    ref = ins["x"].astype(np.float64) @ w.T + b
    rel = np.linalg.norm(got - ref) / np.linalg.norm(ref)
    print("L2 rel err vs fp64 numpy:", rel)
